# revision 21
# baseline (speedup 1.0000x reference)
"""Trainium2 Bass kernel for nn_AttGraphConvLayer.

Reference computation (per batch b):
    z   = nodes @ w                          [N, D]
    z1  = Cmat @ z ; z2 = Nmat @ z           [E, D] (one-hot gathers)
    att = leaky_relu(concat(z1, z2) @ attention)      [E, 1]
    scores = (Cmat^T * att^T) @ Nmat         [N, N]
    adj    = Cmat^T @ Nmat                   [N, N]
    logits = scores + (1 - adj) * (-1e9)
    out = leaky_relu(softmax(logits, -1) * adj @ z)   [N, D]

Key identities used (Cmat/Nmat are one-hot incidence matrices):
  * att_e = leaky(u[src_e] + v[dst_e]) with u = z @ a_top, v = z @ a_bot.
    Hence scores[n, m] = adj[n, m] * leaky(u[n] + v[m]) -- no [E,D]
    gathers and no scores matmul are needed at all; only the adjacency
    matmul (contraction over E) remains.
  * adj has 0/1 inputs, so the adjacency matmul is EXACT in fp8 (e4m3;
    products are 0/1, fp32 PSUM accumulation) and runs ~8x faster than
    fp32 on the PE with perf_mode=DoubleRow (2 edges contracted per
    cell per cycle). The incidence matrices are shipped as fp8 from the
    host (exact, less DMA, no on-device casts).
  * v = z @ a_bot = nodes @ (w @ a_bot): computed via a tiny on-device
    reduction (wb = sum_d w*a_bot) plus a PE matvec against nodes^T.
    nodes^T itself is shipped from the host (layout choice), removing
    all PE transposes from the prologue.

Sharding: 8 cores = 4 batches x 2 row-halves (graph partitioning by
source node). A core's output rows n in [h*512,(h+1)*512) only receive
contributions from edges with src in that range, so the host ships each
core only those ~4096 edges, further grouped by 128-row source chunk
(each group padded with all-zero rows to a fixed 1280). Grouping makes
the one-hot source block only 128 columns wide, so each adjacency PSUM
tile needs just its own group's edges: 40 DoubleRow matmuls total.
All cores run the same program; the host permutes the node axis per core
so the core's 512 output rows are always rows 0..511 (applied
consistently to nodes rows, Cmat columns and Nmat columns; softmax and
the final contraction over the m axis are permutation invariant).
"""

import sys

for _p in ("/opt/trn_rl_repo", "/root/.axon_site/_ro/trn_rl_repo"):
    if _p not in sys.path:
        sys.path.insert(0, _p)

import numpy as np

B, E, N, F, D = 4, 8192, 1024, 512, 512
H = N // 2          # rows per core
P = 128
EPG = 1280          # padded edges per source-chunk group; group size is
                    # Binom(8192, 1/8): mean 1024, sd 30 -> 1280 is +8.5 sd
                    # (asserted at runtime)
NG = H // P         # 4 groups per core
ALPHA = 0.2
NEG = -1.0e9
N_CORES = 8

_compiled = None


def _build():
    import concourse.bacc as bacc
    import concourse.tile as tile
    import concourse.mybir as mybir
    from concourse.masks import make_identity

    dt = mybir.dt
    f32 = dt.float32
    fp8 = dt.float8e4
    Alu = mybir.AluOpType
    Act = mybir.ActivationFunctionType
    DR = mybir.MatmulPerfMode.DoubleRow

    nc = bacc.Bacc("TRN2", target_bir_lowering=False, debug=False,
                   num_devices=N_CORES)

    # edge groups: group r covers source rows r*128..(r+1)*128
    ch = nc.dram_tensor("ch", [NG, EPG, P], fp8, kind="ExternalInput").ap()
    nf = nc.dram_tensor("nf", [NG, EPG, N], fp8, kind="ExternalInput").ap()
    nodesT = nc.dram_tensor("nodesT", [F, N], f32, kind="ExternalInput").ap()
    w = nc.dram_tensor("w", [F, D], f32, kind="ExternalInput").ap()
    atop = nc.dram_tensor("atop", [1, D], f32, kind="ExternalInput").ap()
    abot = nc.dram_tensor("abot", [1, D], f32, kind="ExternalInput").ap()
    out = nc.dram_tensor("out", [H, D], f32, kind="ExternalOutput").ap()

    NC_N = N // P   # 8 node chunks
    NC_F = F // P   # 4 feature chunks
    NC_H = H // P   # 4 row chunks per core
    SG = EPG // P   # 10 sub-chunks of 128 edges per group
    nT_r = nodesT.rearrange("(c p) n -> p c n", p=P)
    w_r = w.rearrange("(c p) d -> p c d", p=P)

    with tile.TileContext(nc) as tc:
        with tc.tile_pool(name="singles", bufs=1) as singles:
            # ---- input loads: z's operands first, chunk-interleaved ----
            nT_sb = singles.tile([P, NC_F, N], f32, name="nT_sb")
            w_sb = singles.tile([P, NC_F, D], f32, name="w_sb")
            for cf in range(NC_F):
                nc.sync.dma_start(out=nT_sb[:, cf, :], in_=nT_r[:, cf, :])
                nc.sync.dma_start(out=w_sb[:, cf, :], in_=w_r[:, cf, :])
            atop_b = singles.tile([P, D], f32, name="atop_b")
            nc.sync.dma_start(out=atop_b, in_=atop.to_broadcast([P, D]))
            abot_b = singles.tile([P, D], f32, name="abot_b")
            nc.sync.dma_start(out=abot_b, in_=abot.to_broadcast([P, D]))
            ident = singles.tile([P, P], f32, name="ident")
            make_identity(nc, ident)
            negc = singles.tile([P, 1], f32, name="negc")
            nc.vector.memset(negc, NEG)

            # edge-group stream loads (issued early, consumed after z)
            cb_sb = singles.tile([P, NG, SG, P], fp8, name="cb_sb")
            nb_sb = singles.tile([P, NG, SG, N], fp8, name="nb_sb")
            for r in range(NC_H):
                nc.sync.dma_start(
                    out=cb_sb[:, r],
                    in_=ch[r].rearrange("(s p) c -> p s c", p=P))
                nc.sync.dma_start(
                    out=nb_sb[:, r],
                    in_=nf[r].rearrange("(s p) c -> p s c", p=P))

            # ---- z = nodes @ w (fp32), contraction pass outermost so the
            # first matmul only needs the first nT/w chunks ----
            z_sb = singles.tile([P, NC_N, D], f32, name="z_sb")
            u_sb = singles.tile([P, NC_H], f32, name="u_sb")
            wb_sb = singles.tile([P, NC_F], f32, name="wb_sb")
            pT_all = singles.tile([P, NC_H, N], f32, name="pT_all")
            v_row = singles.tile([1, N], f32, name="v_row")
            V_bc = singles.tile([P, N], f32, name="V_bc")
            # adjacency PSUM pool first: it gets banks untouched by the z
            # pools, so the early adjacency matmuls never wait on a release
            adj_ps = tc.alloc_tile_pool(name="adj_ps", bufs=2, space="PSUM")

            def emit_adj(r):
                pj = []
                for j in range(2):
                    apj = adj_ps.tile([P, 512], f32,
                                      name=f"adj_{r}_{j}", tag=f"adj_{j}")
                    pj.append(apj)
                    for t in range(SG // 2):
                        ks = slice(2 * t, 2 * t + 2)
                        nc.tensor.matmul(
                            apj,
                            lhsT=cb_sb[:, r, ks, :],
                            rhs=nb_sb[:, r, ks, j * 512:(j + 1) * 512],
                            start=(t == 0), stop=(t == SG // 2 - 1),
                            perf_mode=DR)
                return pj

            with tc.tile_pool(name="uscr", bufs=2) as uscr:
                # ---- z rows 0..511 + v ----
                with tc.tile_pool(name="zA_ps", bufs=1,
                                  space="PSUM") as zA_ps:
                    zpA = [zA_ps.tile([P, D], f32, name=f"zp_{cn}",
                                      tag=f"zp_{cn}") for cn in range(4)]
                    for cf in range(NC_F):
                        for cn in range(4):
                            nc.tensor.matmul(
                                zpA[cn],
                                lhsT=nT_sb[:, cf, cn * P:(cn + 1) * P],
                                rhs=w_sb[:, cf, :],
                                start=(cf == 0), stop=(cf == NC_F - 1))
                    # wb[f] = sum_d w[f,d] * a_bot[d] (overlaps z matmuls)
                    for cf in range(NC_F):
                        ws = uscr.tile([P, D], f32, name=f"ws_{cf}", tag="us")
                        nc.vector.tensor_mul(ws, w_sb[:, cf, :], abot_b)
                        nc.vector.tensor_reduce(
                            wb_sb[:, cf:cf + 1], ws,
                            axis=mybir.AxisListType.X, op=Alu.add)
                    # v[m] = sum_f nodes[m,f] * wb[f]
                    for jm in range(2):
                        vp = zA_ps.tile([1, 512], f32, name=f"vp_{jm}",
                                        tag=f"zp_{jm}")
                        for cf in range(NC_F):
                            nc.tensor.matmul(
                                vp,
                                lhsT=wb_sb[:, cf:cf + 1],
                                rhs=nT_sb[:, cf, jm * 512:(jm + 1) * 512],
                                start=(cf == 0), stop=(cf == NC_F - 1))
                        nc.vector.tensor_copy(
                            v_row[:, jm * 512:(jm + 1) * 512], vp)
                    nc.gpsimd.partition_broadcast(V_bc, v_row)
                    for cn in range(4):
                        if cn % 2 == 0:
                            nc.vector.tensor_copy(z_sb[:, cn, :], zpA[cn])
                        else:
                            nc.scalar.copy(z_sb[:, cn, :], zpA[cn])
                        # u[n] = sum_d z[n,d] * a_top[d]
                        us = uscr.tile([P, D], f32, name=f"us_{cn}",
                                       tag="us")
                        nc.vector.tensor_mul(us, z_sb[:, cn, :], atop_b)
                        nc.vector.tensor_reduce(
                            u_sb[:, cn:cn + 1], us,
                            axis=mybir.AxisListType.X, op=Alu.add)
                    # pT = leaky(u[n] + v[m]) per row chunk
                    for r in range(NC_H):
                        t_uv = uscr.tile([P, N], f32, name=f"tuv_{r}",
                                         tag="tuv")
                        nc.scalar.activation(t_uv, V_bc, Act.Identity,
                                             bias=u_sb[:, r:r + 1],
                                             scale=1.0)
                        nc.vector.scalar_tensor_tensor(
                            out=pT_all[:, r, :], in0=t_uv, scalar=ALPHA,
                            in1=t_uv, op0=Alu.mult, op1=Alu.max)

                # ---- adjacency for row chunks 0/1 fills the PE while the
                # zA pool drains; then z rows 512..1023 reuse zA's banks ----
                adj_tiles = {0: emit_adj(0), 1: emit_adj(1)}
                zB_ps = tc.alloc_tile_pool(name="zB_ps", bufs=1,
                                           space="PSUM")
                zpB = [zB_ps.tile([P, D], f32, name=f"zp_{cn}",
                                  tag=f"zp_{cn}") for cn in range(4, NC_N)]
                for cf in range(NC_F):
                    for cn in range(4, NC_N):
                        nc.tensor.matmul(
                            zpB[cn - 4],
                            lhsT=nT_sb[:, cf, cn * P:(cn + 1) * P],
                            rhs=w_sb[:, cf, :],
                            start=(cf == 0), stop=(cf == NC_F - 1))
                for cn in range(4, NC_N):
                    if cn % 2 == 0:
                        nc.vector.tensor_copy(z_sb[:, cn, :], zpB[cn - 4])
                    else:
                        nc.scalar.copy(z_sb[:, cn, :], zpB[cn - 4])
                zB_ps.release()

            # ---- adjacency matmul + softmax + transpose + out, per r ----
            # adj row-chunk r only needs edge group r (grouped by source).
            # logits = adj*pT + (adj-1)*1e9
            # (exact: for adj==1 the +(adj-1)*1e9 term is exactly 0)
            # softmax pipeline runs in m-halves (j = 0/1) to shorten the
            # serial chain; adjacency stays resident in PSUM.
            eT_sb = singles.tile([P, NC_N, H], f32, name="eT_sb")
            out_r = out.rearrange("(r p) d -> p r d", p=P)
            # software pipeline: adjacency for row chunks r and r+1 in
            # flight while chunk r-2's softmax/transpose/matmul drain
            with tc.tile_pool(name="pscr", bufs=2) as pscr, \
                 tc.tile_pool(name="sml", bufs=6) as sml, \
                 tc.tile_pool(name="tp2_ps", bufs=2, space="PSUM") as tp2_ps, \
                 tc.tile_pool(name="o_ps", bufs=2, space="PSUM") as o_ps, \
                 tc.tile_pool(name="oscr", bufs=2) as oscr:
                for r in range(NC_H):
                    adj_pj = adj_tiles.pop(r)
                    # softmax over m, pipelined in halves (reads PSUM)
                    m1 = pscr.tile([P, N], f32, name=f"m1_{r}", tag="m1")
                    s19 = pscr.tile([P, N], f32, name=f"s19_{r}", tag="s19")
                    lg = pscr.tile([P, N], f32, name=f"lg_{r}", tag="lg")
                    nmh = sml.tile([P, 2], f32, name=f"nmh_{r}", tag="nmh")
                    for j in range(2):
                        sl = slice(j * 512, (j + 1) * 512)
                        nc.vector.tensor_mul(m1[:, sl], adj_pj[j],
                                             pT_all[:, r, sl])
                        # (adj-1)*1e9 via ACT free affine (fma: exact)
                        nc.scalar.activation(s19[:, sl], adj_pj[j],
                                             Act.Identity,
                                             bias=negc[:, 0:1], scale=-NEG)
                        nc.vector.tensor_add(lg[:, sl], m1[:, sl], s19[:, sl])
                        nc.vector.tensor_reduce(
                            nmh[:, j:j + 1], lg[:, sl],
                            axis=mybir.AxisListType.X, op=Alu.max,
                            negate=True)
                    nmx = sml.tile([P, 1], f32, name=f"nmx_{r}", tag="nmx")
                    # -max(a,b) = min(-a,-b)
                    nc.vector.tensor_tensor(nmx, nmh[:, 0:1], nmh[:, 1:2],
                                            op=Alu.min)
                    e_t = pscr.tile([P, N], f32, name=f"et_{r}", tag="et")
                    ea = pscr.tile([P, N], f32, name=f"ea_{r}", tag="ea")
                    zh = sml.tile([P, 2], f32, name=f"zh_{r}", tag="zh")
                    for j in range(2):
                        sl = slice(j * 512, (j + 1) * 512)
                        nc.scalar.activation(e_t[:, sl], lg[:, sl], Act.Exp,
                                             bias=nmx[:, 0:1], scale=1.0,
                                             accum_out=zh[:, j:j + 1])
                        nc.vector.tensor_mul(ea[:, sl], e_t[:, sl], adj_pj[j])
                    zr = sml.tile([P, 1], f32, name=f"zr_{r}", tag="zr")
                    nc.vector.tensor_add(zr, zh[:, 0:1], zh[:, 1:2])
                    rcp = sml.tile([P, 1], f32, name=f"rcp_{r}", tag="rcp")
                    nc.vector.reciprocal(rcp, zr)

                    if r + 2 < NC_H:
                        adj_tiles[r + 2] = emit_adj(r + 2)

                    # transpose expadj chunk into eT columns r*128..
                    for cm in range(NC_N):
                        pt2 = tp2_ps.tile([P, P], f32, name=f"tp2_{r}_{cm}",
                                          tag="tp2")
                        nc.tensor.transpose(
                            pt2, ea[:, cm * P:(cm + 1) * P], ident)
                        if cm % 3 == 0:
                            nc.vector.tensor_copy(
                                eT_sb[:, cm, r * P:(r + 1) * P], pt2)
                        else:
                            nc.scalar.copy(
                                eT_sb[:, cm, r * P:(r + 1) * P], pt2)

                    # out chunk = leaky(rcp * (expadj^T)^T @ z)
                    op = o_ps.tile([P, D], f32, name=f"op_{r}", tag="op")
                    for cm in range(NC_N):
                        nc.tensor.matmul(
                            op,
                            lhsT=eT_sb[:, cm, r * P:(r + 1) * P],
                            rhs=z_sb[:, cm, :],
                            start=(cm == 0), stop=(cm == NC_N - 1))
                    o_t = oscr.tile([P, D], f32, name=f"ot_{r}", tag="ot")
                    nc.scalar.mul(o_t, op, rcp[:, 0:1])
                    o_l = oscr.tile([P, D], f32, name=f"ol_{r}", tag="ol")
                    nc.vector.scalar_tensor_tensor(
                        out=o_l, in0=o_t, scalar=ALPHA, in1=o_t,
                        op0=Alu.mult, op1=Alu.max)
                    nc.sync.dma_start(out=out_r[:, r, :], in_=o_l)
            adj_ps.release()

    nc.compile()
    return nc


def _get_compiled():
    global _compiled
    if _compiled is None:
        _compiled = _build()
    return _compiled


def _in_maps(nodes, Cmat, Nmat, w, attention):
    import ml_dtypes
    f8 = ml_dtypes.float8_e4m3
    nodes = np.asarray(nodes, dtype=np.float32)
    Cmat = np.asarray(Cmat, dtype=np.float32)
    Nmat = np.asarray(Nmat, dtype=np.float32)
    w = np.ascontiguousarray(np.asarray(w, dtype=np.float32))
    attention = np.asarray(attention, dtype=np.float32)
    atop = np.ascontiguousarray(attention[:D, 0][None, :])
    abot = np.ascontiguousarray(attention[D:, 0][None, :])
    maps = []
    for core in range(N_CORES):
        b, h = divmod(core, 2)
        lo, hi = h * H, (h + 1) * H
        src = Cmat[b].argmax(axis=1)
        ch_g = np.zeros((NG, EPG, P), dtype=f8)
        nf_g = np.zeros((NG, EPG, N), dtype=f8)
        for r in range(NG):
            glo = lo + r * P
            sel = np.nonzero((src >= glo) & (src < glo + P))[0]
            assert len(sel) <= EPG, f"group overflow: {len(sel)} > {EPG}"
            ch_g[r, :len(sel)] = Cmat[b][sel][:, glo:glo + P].astype(f8)
            nf_b = Nmat[b][sel]
            if h == 0:
                nf_g[r, :len(sel)] = nf_b.astype(f8)
            else:
                nf_g[r, :len(sel), :H] = nf_b[:, lo:hi].astype(f8)
                nf_g[r, :len(sel), H:] = nf_b[:, :lo].astype(f8)
        if h == 0:
            nodesT = nodes[b].T
        else:
            nodesT = np.concatenate([nodes[b, lo:hi], nodes[b, :lo]],
                                    axis=0).T
        maps.append({
            "ch": ch_g,
            "nf": nf_g,
            "nodesT": np.ascontiguousarray(nodesT),
            "w": w,
            "atop": atop,
            "abot": abot,
        })
    return maps


def kernel(nodes, Cmat, Nmat, mask, w, attention, _trace=False, _tmpdir=None):
    from concourse.bass_utils import run_bass_kernel_spmd

    nc = _get_compiled()
    maps = _in_maps(nodes, Cmat, Nmat, w, attention)
    res = run_bass_kernel_spmd(nc, maps, list(range(N_CORES)),
                               trace=_trace, tmpdir=_tmpdir)
    full = np.empty((B, N, D), dtype=np.float32)
    for core in range(N_CORES):
        b, h = divmod(core, 2)
        full[b, h * H:(h + 1) * H, :] = res.results[core]["out"]
    if _trace:
        return full, res
    return full


if __name__ == "__main__":
    rng = np.random.default_rng(0)
    src = rng.integers(0, N, (B, E))
    dst = rng.integers(0, N, (B, E))
    Cm = np.eye(N, dtype=np.float32)[src]
    Nm = np.eye(N, dtype=np.float32)[dst]
    nodes = rng.standard_normal((B, N, F)).astype(np.float32)
    w = (rng.standard_normal((F, D)) * 0.05).astype(np.float32)
    att = (rng.standard_normal((2 * D, 1)) * 0.05).astype(np.float32)
    mask = np.ones((B, N, N), dtype=bool)
    got = kernel(nodes, Cm, Nm, mask, w, att)
    print("kernel ran, output shape", got.shape)


# revision 22
# speedup vs baseline: 1.1161x; 1.1161x over previous
"""Trainium2 Bass kernel for nn_AttGraphConvLayer.

Reference computation (per batch b):
    z   = nodes @ w                          [N, D]
    z1  = Cmat @ z ; z2 = Nmat @ z           [E, D] (one-hot gathers)
    att = leaky_relu(concat(z1, z2) @ attention)      [E, 1]
    scores = (Cmat^T * att^T) @ Nmat         [N, N]
    adj    = Cmat^T @ Nmat                   [N, N]
    logits = scores + (1 - adj) * (-1e9)
    out = leaky_relu(softmax(logits, -1) * adj @ z)   [N, D]

Key identities used (Cmat/Nmat are one-hot incidence matrices):
  * att_e = leaky(u[src_e] + v[dst_e]) with u = z @ a_top, v = z @ a_bot.
    Hence scores[n, m] = adj[n, m] * leaky(u[n] + v[m]) -- no [E,D]
    gathers and no scores matmul are needed at all; only the adjacency
    matmul (contraction over E) remains.
  * adj has 0/1 inputs, so the adjacency matmul is EXACT in fp8 (e4m3;
    products are 0/1, fp32 PSUM accumulation) and runs ~8x faster than
    fp32 on the PE with perf_mode=DoubleRow (2 edges contracted per
    cell per cycle). The incidence matrices are shipped as fp8 from the
    host (exact, less DMA, no on-device casts).
  * v = z @ a_bot = nodes @ (w @ a_bot): computed via a tiny on-device
    reduction (wb = sum_d w*a_bot) plus a PE matvec against nodes^T.
    nodes^T itself is shipped from the host (layout choice), removing
    all PE transposes from the prologue.

Sharding: 8 cores = 4 batches x 2 row-halves (graph partitioning by
source node). A core's output rows n in [h*512,(h+1)*512) only receive
contributions from edges with src in that range, so the host ships each
core only those ~4096 edges, further grouped by 128-row source chunk
(each group padded with all-zero rows to a fixed 1280). Grouping makes
the one-hot source block only 128 columns wide, so each adjacency PSUM
tile needs just its own group's edges: 40 DoubleRow matmuls total.
All cores run the same program; the host permutes the node axis per core
so the core's 512 output rows are always rows 0..511 (applied
consistently to nodes rows, Cmat columns and Nmat columns; softmax and
the final contraction over the m axis are permutation invariant).
"""

import sys

for _p in ("/opt/trn_rl_repo", "/root/.axon_site/_ro/trn_rl_repo"):
    if _p not in sys.path:
        sys.path.insert(0, _p)

import numpy as np

B, E, N, F, D = 4, 8192, 1024, 512, 512
H = N // 2          # rows per core
P = 128
EPG = 1280          # padded edges per source-chunk group; group size is
                    # Binom(8192, 1/8): mean 1024, sd 30 -> 1280 is +8.5 sd
                    # (asserted at runtime)
NG = H // P         # 4 groups per core
ALPHA = 0.2
NEG = -1.0e9
N_CORES = 8

_compiled = None


def _build():
    import concourse.bacc as bacc
    import concourse.tile as tile
    import concourse.mybir as mybir
    from concourse.masks import make_identity

    dt = mybir.dt
    f32 = dt.float32
    fp8 = dt.float8e4
    Alu = mybir.AluOpType
    Act = mybir.ActivationFunctionType
    DR = mybir.MatmulPerfMode.DoubleRow

    nc = bacc.Bacc("TRN2", target_bir_lowering=False, debug=False,
                   num_devices=N_CORES)

    # edge groups: group r covers source rows r*128..(r+1)*128
    ch = nc.dram_tensor("ch", [NG, EPG, P], fp8, kind="ExternalInput").ap()
    nf = nc.dram_tensor("nf", [NG, EPG, N], fp8, kind="ExternalInput").ap()
    bf16 = dt.bfloat16
    nTh = nc.dram_tensor("nTh", [F, N], bf16, kind="ExternalInput").ap()
    nTl = nc.dram_tensor("nTl", [F, N], bf16, kind="ExternalInput").ap()
    wh = nc.dram_tensor("wh", [F, D], bf16, kind="ExternalInput").ap()
    wl = nc.dram_tensor("wl", [F, D], bf16, kind="ExternalInput").ap()
    atop = nc.dram_tensor("atop", [1, D], f32, kind="ExternalInput").ap()
    abot = nc.dram_tensor("abot", [1, D], f32, kind="ExternalInput").ap()
    out = nc.dram_tensor("out", [H, D], f32, kind="ExternalOutput").ap()

    NC_N = N // P   # 8 node chunks
    NC_F = F // P   # 4 feature chunks
    NC_H = H // P   # 4 row chunks per core
    SG = EPG // P   # 10 sub-chunks of 128 edges per group
    nTh_r = nTh.rearrange("(c p) n -> p c n", p=P)
    nTl_r = nTl.rearrange("(c p) n -> p c n", p=P)
    wh_r = wh.rearrange("(c p) d -> p c d", p=P)
    wl_r = wl.rearrange("(c p) d -> p c d", p=P)

    with tile.TileContext(nc) as tc:
        with tc.tile_pool(name="singles", bufs=1) as singles:
            # ---- input loads: z's operands first, chunk-interleaved ----
            nTh_sb = singles.tile([P, NC_F, N], bf16, name="nTh_sb")
            nTl_sb = singles.tile([P, NC_F, N], bf16, name="nTl_sb")
            wh_sb = singles.tile([P, NC_F, D], bf16, name="wh_sb")
            wl_sb = singles.tile([P, NC_F, D], bf16, name="wl_sb")
            for cf in range(NC_F):
                nc.sync.dma_start(out=nTh_sb[:, cf, :], in_=nTh_r[:, cf, :])
                nc.sync.dma_start(out=wh_sb[:, cf, :], in_=wh_r[:, cf, :])
            for cf in range(NC_F):
                nc.sync.dma_start(out=nTl_sb[:, cf, :], in_=nTl_r[:, cf, :])
                nc.sync.dma_start(out=wl_sb[:, cf, :], in_=wl_r[:, cf, :])
            atop_b = singles.tile([P, D], f32, name="atop_b")
            nc.sync.dma_start(out=atop_b, in_=atop.to_broadcast([P, D]))
            abot_b = singles.tile([P, D], f32, name="abot_b")
            nc.sync.dma_start(out=abot_b, in_=abot.to_broadcast([P, D]))
            ident = singles.tile([P, P], f32, name="ident")
            make_identity(nc, ident)
            negc = singles.tile([P, 1], f32, name="negc")
            nc.vector.memset(negc, NEG)

            # edge-group stream loads (issued early, consumed after z)
            cb_sb = singles.tile([P, NG, SG, P], fp8, name="cb_sb")
            nb_sb = singles.tile([P, NG, SG, N], fp8, name="nb_sb")
            for r in range(NC_H):
                nc.sync.dma_start(
                    out=cb_sb[:, r],
                    in_=ch[r].rearrange("(s p) c -> p s c", p=P))
                nc.sync.dma_start(
                    out=nb_sb[:, r],
                    in_=nf[r].rearrange("(s p) c -> p s c", p=P))

            # ---- z = nodes @ w (fp32), contraction pass outermost so the
            # first matmul only needs the first nT/w chunks ----
            z_sb = singles.tile([P, NC_N, D], f32, name="z_sb")
            u_sb = singles.tile([P, NC_H], f32, name="u_sb")
            wb_sb = singles.tile([P, NC_F], f32, name="wb_sb")
            pT_all = singles.tile([P, NC_H, N], f32, name="pT_all")
            v_row = singles.tile([1, N], f32, name="v_row")
            V_bc = singles.tile([P, N], f32, name="V_bc")
            with tc.tile_pool(name="uscr", bufs=2) as uscr:
                # ---- z rows 0..511 + v, using PSUM banks 0..3 ----
                with tc.tile_pool(name="zA_ps", bufs=1,
                                  space="PSUM") as zA_ps:
                    zpA = [zA_ps.tile([P, D], f32, name=f"zp_{cn}",
                                      tag=f"zp_{cn}") for cn in range(4)]
                    zterms = [(nTh_sb, wh_sb), (nTh_sb, wl_sb),
                              (nTl_sb, wh_sb)]
                    for ti, (nt_t, w_t) in enumerate(zterms):
                        for cf in range(NC_F):
                            for cn in range(4):
                                nc.tensor.matmul(
                                    zpA[cn],
                                    lhsT=nt_t[:, cf, cn * P:(cn + 1) * P],
                                    rhs=w_t[:, cf, :],
                                    start=(ti == 0 and cf == 0),
                                    stop=(ti == 2 and cf == NC_F - 1))
                    # wb[f] = sum_d w[f,d] * a_bot[d] (overlaps z matmuls)
                    wbp = singles.tile([P, NC_F, 2], f32, name="wbp")
                    for cf in range(NC_F):
                        for wi, w_t in enumerate((wh_sb, wl_sb)):
                            ws = uscr.tile([P, D], f32, name=f"ws_{cf}_{wi}",
                                           tag="us")
                            nc.vector.tensor_mul(ws, w_t[:, cf, :], abot_b)
                            nc.vector.tensor_reduce(
                                wbp[:, cf, wi:wi + 1], ws,
                                axis=mybir.AxisListType.X, op=Alu.add)
                        nc.vector.tensor_add(wb_sb[:, cf:cf + 1],
                                             wbp[:, cf, 0:1],
                                             wbp[:, cf, 1:2])
                    # split wb into a bf16 hi/lo pair for the v matvec
                    wbh = singles.tile([P, NC_F], bf16, name="wbh")
                    wbl = singles.tile([P, NC_F], bf16, name="wbl")
                    wbr = singles.tile([P, NC_F], f32, name="wbr")
                    nc.vector.tensor_copy(wbh, wb_sb)
                    nc.vector.tensor_sub(wbr, wb_sb, wbh)
                    nc.vector.tensor_copy(wbl, wbr)
                    # v[m] = sum_f nodes[m,f] * wb[f] (bf16 3-term)
                    vterms = [(wbh, nTh_sb), (wbh, nTl_sb), (wbl, nTh_sb)]
                    for jm in range(2):
                        vp = zA_ps.tile([1, 512], f32, name=f"vp_{jm}",
                                        tag=f"zp_{jm}")
                        for vi, (wb_t, nt_t) in enumerate(vterms):
                            for cf in range(NC_F):
                                nc.tensor.matmul(
                                    vp,
                                    lhsT=wb_t[:, cf:cf + 1],
                                    rhs=nt_t[:, cf,
                                             jm * 512:(jm + 1) * 512],
                                    start=(vi == 0 and cf == 0),
                                    stop=(vi == 2 and cf == NC_F - 1))
                        nc.vector.tensor_copy(
                            v_row[:, jm * 512:(jm + 1) * 512], vp)
                    nc.gpsimd.partition_broadcast(V_bc, v_row)
                    for cn in range(4):
                        if cn % 2 == 0:
                            nc.vector.tensor_copy(z_sb[:, cn, :], zpA[cn])
                        else:
                            nc.scalar.copy(z_sb[:, cn, :], zpA[cn])
                        # u[n] = sum_d z[n,d] * a_top[d]
                        us = uscr.tile([P, D], f32, name=f"us_{cn}",
                                       tag="us")
                        nc.vector.tensor_mul(us, z_sb[:, cn, :], atop_b)
                        nc.vector.tensor_reduce(
                            u_sb[:, cn:cn + 1], us,
                            axis=mybir.AxisListType.X, op=Alu.add)
                        # pT = leaky(u[n] + v[m]) for this row chunk
                        r = cn
                        t_uv = uscr.tile([P, N], f32, name=f"tuv_{r}",
                                         tag="tuv")
                        nc.scalar.activation(t_uv, V_bc, Act.Identity,
                                             bias=u_sb[:, r:r + 1],
                                             scale=1.0)
                        nc.vector.scalar_tensor_tensor(
                            out=pT_all[:, r, :], in0=t_uv, scalar=ALPHA,
                            in1=t_uv, op0=Alu.mult, op1=Alu.max)

                # ---- z rows 512..1023 on the other 4 PSUM banks; the
                # adjacency pool coexists on the banks zA freed, so the
                # adjacency matmuls follow the z matmuls back-to-back ----
                adj_ps = tc.alloc_tile_pool(name="adj_ps", bufs=2,
                                            space="PSUM")
                zB_ps = tc.alloc_tile_pool(name="zB_ps", bufs=1,
                                           space="PSUM")
                zpB = [zB_ps.tile([P, D], f32, name=f"zp_{cn}",
                                  tag=f"zp_{cn}") for cn in range(4, NC_N)]
                for ti, (nt_t, w_t) in enumerate(zterms):
                    for cf in range(NC_F):
                        for cn in range(4, NC_N):
                            nc.tensor.matmul(
                                zpB[cn - 4],
                                lhsT=nt_t[:, cf, cn * P:(cn + 1) * P],
                                rhs=w_t[:, cf, :],
                                start=(ti == 0 and cf == 0),
                                stop=(ti == 2 and cf == NC_F - 1))
                for cn in range(4, NC_N):
                    if cn % 2 == 0:
                        nc.vector.tensor_copy(z_sb[:, cn, :], zpB[cn - 4])
                    else:
                        nc.scalar.copy(z_sb[:, cn, :], zpB[cn - 4])
                zB_ps.release()

            # ---- adjacency matmul + softmax + transpose + out, per r ----
            # adj row-chunk r only needs edge group r (grouped by source).
            # logits = adj*pT + (adj-1)*1e9
            # (exact: for adj==1 the +(adj-1)*1e9 term is exactly 0)
            # softmax pipeline runs in m-halves (j = 0/1) to shorten the
            # serial chain; adjacency stays resident in PSUM.
            eT_sb = singles.tile([P, NC_N, H], f32, name="eT_sb")
            out_r = out.rearrange("(r p) d -> p r d", p=P)

            def emit_adj(r):
                pj = []
                for j in range(2):
                    apj = adj_ps.tile([P, 512], f32,
                                      name=f"adj_{r}_{j}", tag=f"adj_{j}")
                    pj.append(apj)
                    for t in range(SG // 2):
                        ks = slice(2 * t, 2 * t + 2)
                        nc.tensor.matmul(
                            apj,
                            lhsT=cb_sb[:, r, ks, :],
                            rhs=nb_sb[:, r, ks, j * 512:(j + 1) * 512],
                            start=(t == 0), stop=(t == SG // 2 - 1),
                            perf_mode=DR)
                return pj

            # software pipeline: adjacency for row chunks r and r+1 in
            # flight while chunk r-2's softmax/transpose/matmul drain
            adj_tiles = {0: emit_adj(0), 1: emit_adj(1)}
            with tc.tile_pool(name="pscr", bufs=2) as pscr, \
                 tc.tile_pool(name="sml", bufs=6) as sml, \
                 tc.tile_pool(name="tp2_ps", bufs=2, space="PSUM") as tp2_ps, \
                 tc.tile_pool(name="o_ps", bufs=2, space="PSUM") as o_ps, \
                 tc.tile_pool(name="oscr", bufs=2) as oscr:
                for r in range(NC_H):
                    adj_pj = adj_tiles.pop(r)
                    # softmax over m, pipelined in halves (reads PSUM)
                    m1 = pscr.tile([P, N], f32, name=f"m1_{r}", tag="m1")
                    s19 = pscr.tile([P, N], f32, name=f"s19_{r}", tag="s19")
                    lg = pscr.tile([P, N], f32, name=f"lg_{r}", tag="lg")
                    nmh = sml.tile([P, 2], f32, name=f"nmh_{r}", tag="nmh")
                    for j in range(2):
                        sl = slice(j * 512, (j + 1) * 512)
                        nc.vector.tensor_mul(m1[:, sl], adj_pj[j],
                                             pT_all[:, r, sl])
                        # (adj-1)*1e9 via ACT free affine (fma: exact)
                        nc.scalar.activation(s19[:, sl], adj_pj[j],
                                             Act.Identity,
                                             bias=negc[:, 0:1], scale=-NEG)
                        nc.vector.tensor_add(lg[:, sl], m1[:, sl], s19[:, sl])
                        nc.vector.tensor_reduce(
                            nmh[:, j:j + 1], lg[:, sl],
                            axis=mybir.AxisListType.X, op=Alu.max,
                            negate=True)
                    nmx = sml.tile([P, 1], f32, name=f"nmx_{r}", tag="nmx")
                    # -max(a,b) = min(-a,-b)
                    nc.vector.tensor_tensor(nmx, nmh[:, 0:1], nmh[:, 1:2],
                                            op=Alu.min)
                    e_t = pscr.tile([P, N], f32, name=f"et_{r}", tag="et")
                    ea = pscr.tile([P, N], f32, name=f"ea_{r}", tag="ea")
                    zh = sml.tile([P, 2], f32, name=f"zh_{r}", tag="zh")
                    for j in range(2):
                        sl = slice(j * 512, (j + 1) * 512)
                        nc.scalar.activation(e_t[:, sl], lg[:, sl], Act.Exp,
                                             bias=nmx[:, 0:1], scale=1.0,
                                             accum_out=zh[:, j:j + 1])
                        nc.vector.tensor_mul(ea[:, sl], e_t[:, sl], adj_pj[j])
                    zr = sml.tile([P, 1], f32, name=f"zr_{r}", tag="zr")
                    nc.vector.tensor_add(zr, zh[:, 0:1], zh[:, 1:2])
                    rcp = sml.tile([P, 1], f32, name=f"rcp_{r}", tag="rcp")
                    nc.vector.reciprocal(rcp, zr)

                    if r + 2 < NC_H:
                        adj_tiles[r + 2] = emit_adj(r + 2)

                    # transpose expadj chunk into eT columns r*128..
                    for cm in range(NC_N):
                        pt2 = tp2_ps.tile([P, P], f32, name=f"tp2_{r}_{cm}",
                                          tag="tp2")
                        nc.tensor.transpose(
                            pt2, ea[:, cm * P:(cm + 1) * P], ident)
                        if cm % 3 == 0:
                            nc.vector.tensor_copy(
                                eT_sb[:, cm, r * P:(r + 1) * P], pt2)
                        else:
                            nc.scalar.copy(
                                eT_sb[:, cm, r * P:(r + 1) * P], pt2)

                    # out chunk = leaky(rcp * (expadj^T)^T @ z)
                    op = o_ps.tile([P, D], f32, name=f"op_{r}", tag="op")
                    for cm in range(NC_N):
                        nc.tensor.matmul(
                            op,
                            lhsT=eT_sb[:, cm, r * P:(r + 1) * P],
                            rhs=z_sb[:, cm, :],
                            start=(cm == 0), stop=(cm == NC_N - 1))
                    o_t = oscr.tile([P, D], f32, name=f"ot_{r}", tag="ot")
                    nc.scalar.mul(o_t, op, rcp[:, 0:1])
                    o_l = oscr.tile([P, D], f32, name=f"ol_{r}", tag="ol")
                    nc.vector.scalar_tensor_tensor(
                        out=o_l, in0=o_t, scalar=ALPHA, in1=o_t,
                        op0=Alu.mult, op1=Alu.max)
                    nc.sync.dma_start(out=out_r[:, r, :], in_=o_l)
            adj_ps.release()

    nc.compile()
    return nc


def _get_compiled():
    global _compiled
    if _compiled is None:
        _compiled = _build()
    return _compiled


def _in_maps(nodes, Cmat, Nmat, w, attention):
    import ml_dtypes
    f8 = ml_dtypes.float8_e4m3
    bf = ml_dtypes.bfloat16
    nodes = np.asarray(nodes, dtype=np.float32)
    Cmat = np.asarray(Cmat, dtype=np.float32)
    Nmat = np.asarray(Nmat, dtype=np.float32)
    w = np.ascontiguousarray(np.asarray(w, dtype=np.float32))
    wh_a = w.astype(bf)
    wl_a = (w - wh_a.astype(np.float32)).astype(bf)
    attention = np.asarray(attention, dtype=np.float32)
    atop = np.ascontiguousarray(attention[:D, 0][None, :])
    abot = np.ascontiguousarray(attention[D:, 0][None, :])
    maps = []
    for core in range(N_CORES):
        b, h = divmod(core, 2)
        lo, hi = h * H, (h + 1) * H
        src = Cmat[b].argmax(axis=1)
        ch_g = np.zeros((NG, EPG, P), dtype=f8)
        nf_g = np.zeros((NG, EPG, N), dtype=f8)
        for r in range(NG):
            glo = lo + r * P
            sel = np.nonzero((src >= glo) & (src < glo + P))[0]
            assert len(sel) <= EPG, f"group overflow: {len(sel)} > {EPG}"
            ch_g[r, :len(sel)] = Cmat[b][sel][:, glo:glo + P].astype(f8)
            nf_b = Nmat[b][sel]
            if h == 0:
                nf_g[r, :len(sel)] = nf_b.astype(f8)
            else:
                nf_g[r, :len(sel), :H] = nf_b[:, lo:hi].astype(f8)
                nf_g[r, :len(sel), H:] = nf_b[:, :lo].astype(f8)
        if h == 0:
            nodesT = np.ascontiguousarray(nodes[b].T)
        else:
            nodesT = np.ascontiguousarray(
                np.concatenate([nodes[b, lo:hi], nodes[b, :lo]], axis=0).T)
        nTh_a = nodesT.astype(bf)
        nTl_a = (nodesT - nTh_a.astype(np.float32)).astype(bf)
        maps.append({
            "ch": ch_g,
            "nf": nf_g,
            "nTh": nTh_a,
            "nTl": nTl_a,
            "wh": wh_a,
            "wl": wl_a,
            "atop": atop,
            "abot": abot,
        })
    return maps


def kernel(nodes, Cmat, Nmat, mask, w, attention, _trace=False, _tmpdir=None):
    from concourse.bass_utils import run_bass_kernel_spmd

    nc = _get_compiled()
    maps = _in_maps(nodes, Cmat, Nmat, w, attention)
    res = run_bass_kernel_spmd(nc, maps, list(range(N_CORES)),
                               trace=_trace, tmpdir=_tmpdir)
    full = np.empty((B, N, D), dtype=np.float32)
    for core in range(N_CORES):
        b, h = divmod(core, 2)
        full[b, h * H:(h + 1) * H, :] = res.results[core]["out"]
    if _trace:
        return full, res
    return full


if __name__ == "__main__":
    rng = np.random.default_rng(0)
    src = rng.integers(0, N, (B, E))
    dst = rng.integers(0, N, (B, E))
    Cm = np.eye(N, dtype=np.float32)[src]
    Nm = np.eye(N, dtype=np.float32)[dst]
    nodes = rng.standard_normal((B, N, F)).astype(np.float32)
    w = (rng.standard_normal((F, D)) * 0.05).astype(np.float32)
    att = (rng.standard_normal((2 * D, 1)) * 0.05).astype(np.float32)
    mask = np.ones((B, N, N), dtype=bool)
    got = kernel(nodes, Cm, Nm, mask, w, att)
    print("kernel ran, output shape", got.shape)
